# revision 66
# baseline (speedup 1.0000x reference)
"""Trainium2 Bass kernel for nn_CPCircuitLayer (bf16 pipeline).

out[b] = (X_b @ seq_W.T) @ diag(cp) @ (X_b.T @ hid_W.T).T, sharded as
8 cores = (batch, seq-half). Raw Bass with manual semaphores; all HBM
traffic in bf16, PSUM accumulation f32.
"""

import numpy as np

B, S, H, R = 4, 1024, 1024, 32
N_CORES = 8
SH = S // 2
KT = S // 128
MT = SH // 128

_compiled = {}


def _np_fallback(hidden_states, all_indices, seq_W, hid_W, cp_weight):
    seq_emb = np.einsum("bsh,rh->bsr", hidden_states, seq_W)
    hid_emb = np.einsum("bsh,rs->bhr", hidden_states, hid_W)
    s_idx = all_indices[:, 0].astype(np.int64)
    h_idx = all_indices[:, 1].astype(np.int64)
    g_seq = seq_emb[:, s_idx, :]
    g_hid = hid_emb[:, h_idx, :]
    out = np.einsum("bnr,bnr,r->bn", g_seq, g_hid, cp_weight[0])
    return out.reshape(B, S, H).astype(np.float32)


def _bf16(a):
    import ml_dtypes
    return np.ascontiguousarray(np.asarray(a, dtype=np.float32)).astype(
        ml_dtypes.bfloat16)


def _wtile(w):
    return np.ascontiguousarray(
        w.reshape(KT, 128, R).transpose(1, 0, 2).reshape(128, KT * R))


def build_raw_program():
    import contextlib

    import concourse.bass as bass
    import concourse.mybir as mybir

    f32 = mybir.dt.float32
    bf16 = mybir.dt.bfloat16

    nc = bass.Bass("TRN2", target_bir_lowering=False, debug=False,
                   num_devices=N_CORES, enable_partition_id=False)

    x_d = nc.dram_tensor("x", [S, H], bf16, kind="ExternalInput")
    xtp_d = nc.dram_tensor("xtp", [(KT // 2) * 128, 1024], bf16,
                           kind="ExternalInput")
    w_d = nc.dram_tensor("w", [128, 2 * KT * R], bf16, kind="ExternalInput")
    out_d = nc.dram_tensor("out", [SH, H], bf16, kind="ExternalOutput")

    with contextlib.ExitStack() as _xs:
        E = _xs.enter_context
        w_t = E(nc.sbuf_tensor([128, 2 * KT * R], bf16))
        x_t = E(nc.sbuf_tensor([128, KT, H], bf16))
        xt_t = E(nc.sbuf_tensor([128, KT // 2, 1024], bf16))
        hid_sb = E(nc.sbuf_tensor([R, H], bf16))
        scr_sb = E(nc.sbuf_tensor([128, R], bf16))   # ACT-preload scratch
        seq_sb = E(nc.sbuf_tensor([R, SH], bf16))
        o_sb = E(nc.sbuf_tensor([128, MT, H], bf16))
        # all 8 PSUM banks as [128, 512] tensors: the hid/seq factor matmuls
        # use only partitions 0:R of banks 5-7; once their SBUF copies land,
        # the final burst reuses those banks, giving 8 distinct output tiles
        # with no write-after-read rotation stalls
        o_ps = [E(nc.psum_tensor(f"o_ps{i}", [128, 512], f32))
                for i in range(8)]
        hid_ps0, hid_ps1, seq_ps = o_ps[5], o_ps[6], o_ps[7]
        dma_sem = E(nc.semaphore("dma_sem"))
        warm_sem = E(nc.semaphore("warm_sem"))
        w_sem = E(nc.semaphore("w_sem"))
        pe_sem = E(nc.semaphore("pe_sem"))
        dve_sem = E(nc.semaphore("dve_sem"))
        act_sem = E(nc.semaphore("act_sem"))
        x_sem = [E(nc.semaphore(f"x_sem{j}")) for j in range(KT)]
        xtp_sem = [E(nc.semaphore(f"xtp_sem{j}")) for j in range(KT // 2)]
        block = E(nc.Block(no_gpsimd_drain=True))

        sw = lambda k: w_t.ap()[:, k * R:(k + 1) * R]
        hw = lambda k: w_t.ap()[:, KT * R + k * R:KT * R + (k + 1) * R]
        xt = lambda k: xt_t.ap()[:, k // 2, (k % 2) * 512:(k % 2) * 512 + 512]

        @block.sync
        def _(sync):
            for k in range(KT):
                sync.dma_start(
                    out=x_t.ap()[:, k, :],
                    in_=x_d[k * 128:(k + 1) * 128, :],
                ).then_inc(x_sem[k], 16)
            for m in (0, 2):
                sync.wait_ge(dve_sem, 7 + m)
                sync.wait_ge(act_sem, 3 + m)
                sync.dma_start(
                    out=out_d[m * 128:(m + 1) * 128, :],
                    in_=o_sb.ap()[:, m, :],
                ).then_inc(dma_sem, 16)
            sync.wait_ge(dma_sem, 16 * MT)

        @block.tensor
        def _(tensor):
            def dummy(n=512):
                nc.tensor.matmul(o_ps[4].ap()[0:R, 0:n], o_sb.ap()[:, 3, 0:R],
                                 o_sb.ap()[:, 3, 0:n], start=True, stop=True)

            # warm the PE DVFS clock from body start: the operand region is
            # memset by DVE first, so no uninitialized SBUF is ever read.
            # N=256 quantizes the warm-up finely so the overshoot past
            # w-arrival stays small while the busy window never lapses.
            tensor.wait_ge(warm_sem, 1)
            for _ in range(21):
                dummy(256)
            tensor.wait_ge(w_sem, 16)

            def hid(k):
                tensor.wait_ge(x_sem[k], 16)
                for n, ps in enumerate((hid_ps0, hid_ps1)):
                    nc.tensor.matmul(
                        ps.ap()[0:R, :],
                        hw(k), x_t.ap()[:, k, n * 512:(n + 1) * 512],
                        start=(k == 0), stop=(k == KT - 1),
                    ).then_inc(pe_sem, 1)

            def seq(p):
                tensor.wait_ge(xtp_sem[p], 16)
                for k in (2 * p, 2 * p + 1):
                    nc.tensor.matmul(
                        seq_ps.ap()[0:R, :], sw(k), xt(k),
                        start=(k == 0), stop=(k == KT - 1),
                    ).then_inc(pe_sem, 1)

            # hid finishes early (n0 @21, n1 @22) so its copies overlap the
            # seq tail; seq completes @24
            hid(0); seq(0); hid(1)
            hid(2); seq(1); hid(3)
            hid(4); seq(2); hid(5)
            hid(6); hid(7); seq(3)

            for j in range(2 * MT):
                m, n = divmod(j, 2)
                tensor.wait_ge(dve_sem, 3 + m)   # hid q0,q1 + seq chunk m
                if n == 1:
                    tensor.wait_ge(act_sem, 2)   # hid q2,q3
                # bank-reuse WAR: j=5 -> hid_ps0 (freed by dve>=2, implied),
                # j=6 -> hid_ps1 (freed by act>=2), j=7 -> seq_ps (dve>=6,
                # implied by dve>=3+m for m=3)
                if j == 6:
                    tensor.wait_ge(act_sem, 2)
                nc.tensor.matmul(
                    o_ps[j].ap(),
                    seq_sb.ap()[:, m * 128:(m + 1) * 128],
                    hid_sb.ap()[:, n * 512:(n + 1) * 512],
                    start=True, stop=True,
                ).then_inc(pe_sem, 1)

        @block.vector
        def _(vector):
            # initialize the dummy-matmul operand region so the PE warm-up
            # never touches uninitialized SBUF
            nc.vector.memset(o_sb.ap()[:, 3, 0:512], 1.0).then_inc(warm_sem, 1)
            # hid n0 half (PSUM bank 0) is DVE's alone — never read one PSUM
            # bank from two engines at once
            vector.wait_ge(pe_sem, 21)   # hid n0 half complete
            nc.vector.tensor_copy(
                hid_sb.ap()[:, 0:256],
                hid_ps0.ap()[0:R, 0:256]).then_inc(dve_sem, 1)
            nc.vector.tensor_copy(
                hid_sb.ap()[:, 256:512],
                hid_ps0.ap()[0:R, 256:512]).then_inc(dve_sem, 1)
            vector.wait_ge(pe_sem, 24)   # seq complete
            for m in range(MT):
                nc.vector.tensor_copy(
                    seq_sb.ap()[:, m * 128:(m + 1) * 128],
                    seq_ps.ap()[0:R, m * 128:(m + 1) * 128],
                ).then_inc(dve_sem, 1)
            for m in range(MT):   # out n0 copies
                vector.wait_ge(pe_sem, 25 + 2 * m)
                nc.vector.tensor_copy(
                    o_sb.ap()[:, m, 0:512],
                    o_ps[2 * m].ap(),
                ).then_inc(dve_sem, 1)

        @block.scalar
        def _(scalar):
            # w leads the scalar queue so its spin-up overlaps the x queue's
            scalar.dma_start(out=w_t.ap(), in_=w_d[:]).then_inc(w_sem, 16)
            for p in range(KT // 2):
                scalar.dma_start(
                    out=xt_t.ap()[:, p, :],
                    in_=xtp_d[p * 128:(p + 1) * 128, :],
                ).then_inc(xtp_sem[p], 16)
            # dummy copy pulls the lazy ACT table load off the critical path;
            # reads the memset region, writes private scratch (no WAW races)
            scalar.wait_ge(warm_sem, 1)
            nc.scalar.copy(scr_sb.ap(), o_sb.ap()[:, 3, 0:R])
            # hid n1 half (PSUM bank 1) is ACT's alone
            scalar.wait_ge(pe_sem, 22)
            nc.scalar.copy(hid_sb.ap()[:, 512:768],
                           hid_ps1.ap()[0:R, 0:256]).then_inc(act_sem, 1)
            nc.scalar.copy(hid_sb.ap()[:, 768:1024],
                           hid_ps1.ap()[0:R, 256:512]).then_inc(act_sem, 1)
            for m in range(MT):
                scalar.wait_ge(pe_sem, 26 + 2 * m)
                nc.scalar.copy(
                    o_sb.ap()[:, m, 512:1024],
                    o_ps[2 * m + 1].ap(),
                ).then_inc(act_sem, 1)
                if m % 2 == 1:
                    scalar.wait_ge(dve_sem, 7 + m)
                    scalar.wait_ge(act_sem, 3 + m)
                    scalar.dma_start(
                        out=out_d[m * 128:(m + 1) * 128, :],
                        in_=o_sb.ap()[:, m, :],
                    ).then_inc(dma_sem, 16)

    return nc


def _get_program():
    if "nc" not in _compiled:
        _compiled["nc"] = build_raw_program()
    return _compiled["nc"]


def _make_in_maps(hidden_states, seq_W, hid_W, cp_weight):
    swT = _wtile(np.ascontiguousarray(seq_W.T))
    hwT_rows = np.ascontiguousarray((hid_W * cp_weight[0][:, None]).T)
    w_rot = [
        _bf16(np.concatenate([swT, _wtile(np.concatenate(
            [hwT_rows[half * SH:], hwT_rows[:half * SH]], axis=0))], axis=1))
        for half in range(2)
    ]
    in_maps = []
    for c in range(N_CORES):
        b, half = divmod(c, 2)
        xb = _bf16(hidden_states[b])
        if half:
            xb = np.ascontiguousarray(
                np.concatenate([xb[SH:], xb[:SH]], axis=0))
        xt_full = np.ascontiguousarray(xb[:SH, :].T)        # [H, SH]
        xtp = np.ascontiguousarray(
            xt_full.reshape(KT // 2, 2, 128, SH).transpose(0, 2, 1, 3)
            .reshape((KT // 2) * 128, 1024))
        in_maps.append({"x": xb, "xtp": xtp, "w": w_rot[half]})
    return in_maps


def kernel(hidden_states, all_indices, seq_W, hid_W, cp_weight):
    hidden_states = np.asarray(hidden_states, dtype=np.float32)
    seq_W = np.asarray(seq_W, dtype=np.float32)
    hid_W = np.asarray(hid_W, dtype=np.float32)
    cp_weight = np.asarray(cp_weight, dtype=np.float32)
    idx = np.asarray(all_indices)

    n = np.arange(S * H, dtype=idx.dtype)
    if idx.shape != (S * H, 2) or not (
        np.array_equal(idx[:, 0], n // H) and np.array_equal(idx[:, 1], n % H)
    ):
        return _np_fallback(hidden_states, idx, seq_W, hid_W, cp_weight)

    from concourse.bass_utils import run_bass_kernel_spmd

    nc = _get_program()
    in_maps = _make_in_maps(hidden_states, seq_W, hid_W, cp_weight)
    res = run_bass_kernel_spmd(nc, in_maps, list(range(N_CORES)))

    out = np.empty((B, S, H), dtype=np.float32)
    for c in range(N_CORES):
        b, half = divmod(c, 2)
        out[b, half * SH:(half + 1) * SH, :] = np.asarray(
            res.results[c]["out"], dtype=np.float32)
    return out


# revision 68
# speedup vs baseline: 1.1066x; 1.1066x over previous
"""Trainium2 Bass kernel for nn_CPCircuitLayer (bf16 pipeline).

out[b] = (X_b @ seq_W.T) @ diag(cp) @ (X_b.T @ hid_W.T).T, sharded as
8 cores = (batch, seq-half). Raw Bass with manual semaphores; all HBM
traffic in bf16, PSUM accumulation f32.
"""

import numpy as np

B, S, H, R = 4, 1024, 1024, 32
N_CORES = 8
SH = S // 2
KT = S // 128
MT = SH // 128

_compiled = {}


def _np_fallback(hidden_states, all_indices, seq_W, hid_W, cp_weight):
    seq_emb = np.einsum("bsh,rh->bsr", hidden_states, seq_W)
    hid_emb = np.einsum("bsh,rs->bhr", hidden_states, hid_W)
    s_idx = all_indices[:, 0].astype(np.int64)
    h_idx = all_indices[:, 1].astype(np.int64)
    g_seq = seq_emb[:, s_idx, :]
    g_hid = hid_emb[:, h_idx, :]
    out = np.einsum("bnr,bnr,r->bn", g_seq, g_hid, cp_weight[0])
    return out.reshape(B, S, H).astype(np.float32)


def _bf16(a):
    import ml_dtypes
    return np.ascontiguousarray(np.asarray(a, dtype=np.float32)).astype(
        ml_dtypes.bfloat16)


def _wtile(w):
    return np.ascontiguousarray(
        w.reshape(KT, 128, R).transpose(1, 0, 2).reshape(128, KT * R))


def build_raw_program():
    import contextlib

    import concourse.bass as bass
    import concourse.mybir as mybir

    f32 = mybir.dt.float32
    bf16 = mybir.dt.bfloat16

    nc = bass.Bass("TRN2", target_bir_lowering=False, debug=False,
                   num_devices=N_CORES, enable_partition_id=False)

    x_d = nc.dram_tensor("x", [S, H], bf16, kind="ExternalInput")
    xtp_d = nc.dram_tensor("xtp", [(KT // 2) * 128, 1024], bf16,
                           kind="ExternalInput")
    w_d = nc.dram_tensor("w", [128, 2 * KT * R], bf16, kind="ExternalInput")
    out_d = nc.dram_tensor("out", [SH, H], bf16, kind="ExternalOutput")

    with contextlib.ExitStack() as _xs:
        E = _xs.enter_context
        w_t = E(nc.sbuf_tensor([128, 2 * KT * R], bf16))
        x_t = E(nc.sbuf_tensor([128, KT, H], bf16))
        xt_t = E(nc.sbuf_tensor([128, KT // 2, 1024], bf16))
        hid_sb = E(nc.sbuf_tensor([R, H], bf16))
        scr_sb = E(nc.sbuf_tensor([128, R], bf16))   # ACT-preload scratch
        seq_sb = E(nc.sbuf_tensor([R, SH], bf16))
        o_sb = E(nc.sbuf_tensor([128, MT, H], bf16))
        # all 8 PSUM banks as [128, 512] tensors: the hid/seq factor matmuls
        # use only partitions 0:R of banks 5-7; once their SBUF copies land,
        # the final burst reuses those banks, giving 8 distinct output tiles
        # with no write-after-read rotation stalls
        o_ps = [E(nc.psum_tensor(f"o_ps{i}", [128, 512], f32))
                for i in range(8)]
        hid_ps0, hid_ps1, seq_ps = o_ps[5], o_ps[6], o_ps[7]
        dma_sem = E(nc.semaphore("dma_sem"))
        warm_sem = E(nc.semaphore("warm_sem"))
        w_sem = E(nc.semaphore("w_sem"))
        pe_sem = E(nc.semaphore("pe_sem"))
        dve_sem = E(nc.semaphore("dve_sem"))
        act_sem = E(nc.semaphore("act_sem"))
        x_sem = [E(nc.semaphore(f"x_sem{j}")) for j in range(KT)]
        xtp_sem = [E(nc.semaphore(f"xtp_sem{j}")) for j in range(KT // 2)]
        block = E(nc.Block(no_gpsimd_drain=True))

        sw = lambda k: w_t.ap()[:, k * R:(k + 1) * R]
        hw = lambda k: w_t.ap()[:, KT * R + k * R:KT * R + (k + 1) * R]
        xt = lambda k: xt_t.ap()[:, k // 2, (k % 2) * 512:(k % 2) * 512 + 512]

        @block.sync
        def _(sync):
            for k in range(KT):
                sync.dma_start(
                    out=x_t.ap()[:, k, :],
                    in_=x_d[k * 128:(k + 1) * 128, :],
                ).then_inc(x_sem[k], 16)
            # sync is idle by the tail — it dispatches all four out DMAs so
            # the ACT engine stays a pure copier
            for m in range(MT):
                sync.wait_ge(dve_sem, 7 + m)
                sync.wait_ge(act_sem, 3 + m)
                sync.dma_start(
                    out=out_d[m * 128:(m + 1) * 128, :],
                    in_=o_sb.ap()[:, m, :],
                ).then_inc(dma_sem, 16)
            sync.wait_ge(dma_sem, 16 * MT)

        @block.tensor
        def _(tensor):
            def dummy(n=512):
                nc.tensor.matmul(o_ps[4].ap()[0:R, 0:n], o_sb.ap()[:, 3, 0:R],
                                 o_sb.ap()[:, 3, 0:n], start=True, stop=True)

            # warm the PE DVFS clock from body start: the operand region is
            # memset by DVE first, so no uninitialized SBUF is ever read.
            # N=256 quantizes the warm-up finely so the overshoot past
            # w-arrival stays small while the busy window never lapses.
            tensor.wait_ge(warm_sem, 1)
            for _ in range(21):
                dummy(256)
            tensor.wait_ge(w_sem, 16)

            def hid(k):
                tensor.wait_ge(x_sem[k], 16)
                for n, ps in enumerate((hid_ps0, hid_ps1)):
                    nc.tensor.matmul(
                        ps.ap()[0:R, :],
                        hw(k), x_t.ap()[:, k, n * 512:(n + 1) * 512],
                        start=(k == 0), stop=(k == KT - 1),
                    ).then_inc(pe_sem, 1)

            def seq(p):
                tensor.wait_ge(xtp_sem[p], 16)
                for k in (2 * p, 2 * p + 1):
                    nc.tensor.matmul(
                        seq_ps.ap()[0:R, :], sw(k), xt(k),
                        start=(k == 0), stop=(k == KT - 1),
                    ).then_inc(pe_sem, 1)

            # hid finishes early (n0 @21, n1 @22) so its copies overlap the
            # seq tail; seq completes @24
            hid(0); seq(0); hid(1)
            hid(2); seq(1); hid(3)
            hid(4); seq(2); hid(5)
            hid(6); hid(7); seq(3)

            for j in range(2 * MT):
                m, n = divmod(j, 2)
                tensor.wait_ge(dve_sem, 3 + m)   # hid q0,q1 + seq chunk m
                if n == 1:
                    tensor.wait_ge(act_sem, 2)   # hid q2,q3
                # bank-reuse WAR: j=5 -> hid_ps0 (freed by dve>=2, implied),
                # j=6 -> hid_ps1 (freed by act>=2), j=7 -> seq_ps (dve>=6,
                # implied by dve>=3+m for m=3)
                if j == 6:
                    tensor.wait_ge(act_sem, 2)
                nc.tensor.matmul(
                    o_ps[j].ap(),
                    seq_sb.ap()[:, m * 128:(m + 1) * 128],
                    hid_sb.ap()[:, n * 512:(n + 1) * 512],
                    start=True, stop=True,
                ).then_inc(pe_sem, 1)

        @block.vector
        def _(vector):
            # initialize the dummy-matmul operand region so the PE warm-up
            # never touches uninitialized SBUF
            nc.vector.memset(o_sb.ap()[:, 3, 0:512], 1.0).then_inc(warm_sem, 1)
            # hid n0 half (PSUM bank 0) is DVE's alone — never read one PSUM
            # bank from two engines at once
            vector.wait_ge(pe_sem, 21)   # hid n0 half complete
            nc.vector.tensor_copy(
                hid_sb.ap()[:, 0:256],
                hid_ps0.ap()[0:R, 0:256]).then_inc(dve_sem, 1)
            nc.vector.tensor_copy(
                hid_sb.ap()[:, 256:512],
                hid_ps0.ap()[0:R, 256:512]).then_inc(dve_sem, 1)
            vector.wait_ge(pe_sem, 24)   # seq complete
            for m in range(MT):
                nc.vector.tensor_copy(
                    seq_sb.ap()[:, m * 128:(m + 1) * 128],
                    seq_ps.ap()[0:R, m * 128:(m + 1) * 128],
                ).then_inc(dve_sem, 1)
            for m in range(MT):   # out n0 copies
                vector.wait_ge(pe_sem, 25 + 2 * m)
                nc.vector.tensor_copy(
                    o_sb.ap()[:, m, 0:512],
                    o_ps[2 * m].ap(),
                ).then_inc(dve_sem, 1)

        @block.scalar
        def _(scalar):
            # w leads the scalar queue so its spin-up overlaps the x queue's
            scalar.dma_start(out=w_t.ap(), in_=w_d[:]).then_inc(w_sem, 16)
            for p in range(KT // 2):
                scalar.dma_start(
                    out=xt_t.ap()[:, p, :],
                    in_=xtp_d[p * 128:(p + 1) * 128, :],
                ).then_inc(xtp_sem[p], 16)
            # dummy copy pulls the lazy ACT table load off the critical path;
            # reads the memset region, writes private scratch (no WAW races)
            scalar.wait_ge(warm_sem, 1)
            nc.scalar.copy(scr_sb.ap(), o_sb.ap()[:, 3, 0:R])
            # hid n1 half (PSUM bank 1) is ACT's alone
            scalar.wait_ge(pe_sem, 22)
            nc.scalar.copy(hid_sb.ap()[:, 512:768],
                           hid_ps1.ap()[0:R, 0:256]).then_inc(act_sem, 1)
            nc.scalar.copy(hid_sb.ap()[:, 768:1024],
                           hid_ps1.ap()[0:R, 256:512]).then_inc(act_sem, 1)
            for m in range(MT):
                scalar.wait_ge(pe_sem, 26 + 2 * m)
                nc.scalar.copy(
                    o_sb.ap()[:, m, 512:1024],
                    o_ps[2 * m + 1].ap(),
                ).then_inc(act_sem, 1)

    return nc


def _get_program():
    if "nc" not in _compiled:
        _compiled["nc"] = build_raw_program()
    return _compiled["nc"]


def _make_in_maps(hidden_states, seq_W, hid_W, cp_weight):
    swT = _wtile(np.ascontiguousarray(seq_W.T))
    hwT_rows = np.ascontiguousarray((hid_W * cp_weight[0][:, None]).T)
    w_rot = [
        _bf16(np.concatenate([swT, _wtile(np.concatenate(
            [hwT_rows[half * SH:], hwT_rows[:half * SH]], axis=0))], axis=1))
        for half in range(2)
    ]
    in_maps = []
    for c in range(N_CORES):
        b, half = divmod(c, 2)
        xb = _bf16(hidden_states[b])
        if half:
            xb = np.ascontiguousarray(
                np.concatenate([xb[SH:], xb[:SH]], axis=0))
        xt_full = np.ascontiguousarray(xb[:SH, :].T)        # [H, SH]
        xtp = np.ascontiguousarray(
            xt_full.reshape(KT // 2, 2, 128, SH).transpose(0, 2, 1, 3)
            .reshape((KT // 2) * 128, 1024))
        in_maps.append({"x": xb, "xtp": xtp, "w": w_rot[half]})
    return in_maps


def kernel(hidden_states, all_indices, seq_W, hid_W, cp_weight):
    hidden_states = np.asarray(hidden_states, dtype=np.float32)
    seq_W = np.asarray(seq_W, dtype=np.float32)
    hid_W = np.asarray(hid_W, dtype=np.float32)
    cp_weight = np.asarray(cp_weight, dtype=np.float32)
    idx = np.asarray(all_indices)

    n = np.arange(S * H, dtype=idx.dtype)
    if idx.shape != (S * H, 2) or not (
        np.array_equal(idx[:, 0], n // H) and np.array_equal(idx[:, 1], n % H)
    ):
        return _np_fallback(hidden_states, idx, seq_W, hid_W, cp_weight)

    from concourse.bass_utils import run_bass_kernel_spmd

    nc = _get_program()
    in_maps = _make_in_maps(hidden_states, seq_W, hid_W, cp_weight)
    res = run_bass_kernel_spmd(nc, in_maps, list(range(N_CORES)))

    out = np.empty((B, S, H), dtype=np.float32)
    for c in range(N_CORES):
        b, half = divmod(c, 2)
        out[b, half * SH:(half + 1) * SH, :] = np.asarray(
            res.results[c]["out"], dtype=np.float32)
    return out
